# revision 3
# baseline (speedup 1.0000x reference)
"""GQA attention kernel for 8 Trainium2 NeuronCores (Bass/Tile).

Sharding: data-parallel over batch (2) x tensor-parallel over head groups (4).
Core c: batch b=c//4, group g=c%4 (query heads 4g..4g+3, kv head g).
w_q/w_k/w_v column-parallel, w_o row-parallel; partial outputs are
ReduceScattered on-device over groups [[0..3],[4..7]]; host gather is a pure
concatenation.

Hardcoded problem: B=2 T=2048 D=1024 n_heads=16 n_kv=4 d_head=64, causal,
RoPE theta=1e4 (freqs passed as input), scale=1/8.
"""

import numpy as np

import concourse.bass as bass
import concourse.tile as tile
from concourse import bacc, mybir
from concourse.bass_utils import run_bass_kernel_spmd
from concourse.masks import make_identity

F32 = mybir.dt.float32
BF16 = mybir.dt.bfloat16

B, T, D = 2, 2048, 1024
NH, NKV, DH = 16, 4, 64
HPC = NH // NKV          # query heads per core = 4
OC = HPC * DH            # per-core attn feature cols = 256
TB = T // 128            # 16 blocks of 128 rows
NJ = T // 512            # 4 tq-slices of 512
GROUPS = [[0, 1, 2, 3], [4, 5, 6, 7]]
SCALE = 1.0 / 8.0

_CACHE = {}


def _emit(nc, tc, aps):
    x_ap, wq_ap, wk_ap, wv_ap, wo_ap, rope_ap, out_ap = aps
    import contextlib
    ctx = contextlib.ExitStack()
    with ctx:
        sing = ctx.enter_context(tc.tile_pool(name="sing", bufs=1))
        stage = ctx.enter_context(tc.tile_pool(name="stage", bufs=3))
        bstage = ctx.enter_context(tc.tile_pool(name="bstage", bufs=3))
        ropet = ctx.enter_context(tc.tile_pool(name="ropet", bufs=8))
        qrp = ctx.enter_context(tc.tile_pool(name="qrp", bufs=3))
        ptp = ctx.enter_context(tc.tile_pool(name="ptp", bufs=4))
        onatp = ctx.enter_context(tc.tile_pool(name="onatp", bufs=8))
        outsbp = ctx.enter_context(tc.tile_pool(name="outsbp", bufs=3))
        rcp = ctx.enter_context(tc.tile_pool(name="rcp", bufs=8))
        # PSUM pools: trst(3) + qkv(2) + oa(2) + outps(1) = 8 banks
        trstp = ctx.enter_context(tc.tile_pool(name="trstp", bufs=3, space="PSUM"))
        qkvp = ctx.enter_context(tc.tile_pool(name="qkvp", bufs=1, space="PSUM"))
        oap = ctx.enter_context(tc.tile_pool(name="oap", bufs=4, space="PSUM"))
        dram = ctx.enter_context(tc.tile_pool(name="dram", bufs=1, space="DRAM"))

        # ---- warm-up collective: pays the rendezvous cost concurrently with compute
        d_in = dram.tile([1, 64], F32)
        d_out = dram.tile([1, 64], F32)
        zt = sing.tile([1, 64], F32)
        nc.vector.memset(zt[:], 0.0)
        nc.sync.dma_start(d_in[:], zt[:])
        nc.gpsimd.collective_compute(
            "AllReduce", mybir.AluOpType.add, replica_groups=GROUPS,
            ins=[d_in.opt()], outs=[d_out.opt()])

        # ---- persistent SBUF tensors
        identb = sing.tile([128, 128], BF16)
        make_identity(nc, identb[:])
        xT = sing.tile([128, 8, T], BF16)        # [d-chunk part, chunk, t]
        wT = sing.tile([128, 8, 384], BF16)      # cols: 0:256 wq | 256:320 wk | 320:384 wv
        woT = sing.tile([128, 2, D], BF16)       # [o-chunk part, chunk, dout]
        qT = sing.tile([64, 4, T], BF16)         # [d_head part, head, t]
        kT = sing.tile([64, T], BF16)
        vaug = sing.tile([128, TB, 65], BF16)    # col 64 = ones (rowsum trick)
        oT = sing.tile([128, 2, T], BF16)
        rope_sb = sing.tile([128, TB, 5, 64], F32)
        nc.vector.memset(vaug[:], 1.0)
        _r = rope_ap.rearrange("(tb p) f -> p tb f", p=128)
        for _tb in range(TB):
            _rt = _r[:, _tb, :]
            _r5 = bass.AP(tensor=_rt.tensor, offset=_rt.offset,
                          ap=[_rt.ap[0], [0, 5], _rt.ap[1]])
            nc.sync.dma_start(rope_sb[:, _tb, :, :], _r5)

        def cast_transpose(src_ap, n_d, dst_view):
            """src [p,128*n_d] bf16 sbuf -> transposed chunks into dst_view(d)."""
            for dch in range(n_d):
                tr = trstp.tile([128, 512], BF16, tag="trst")
                nc.tensor.transpose(tr[:, :128], src_ap[:, 128 * dch:128 * dch + 128],
                                    identb[:])
                dst, pn = dst_view(dch)
                srcv = tr[:pn, :128] if pn < 128 else tr[:, :128]
                if dch % 3 != 2:
                    nc.vector.tensor_copy(dst, srcv)
                else:
                    nc.scalar.copy(dst, srcv)

        # ---- weights: load, cast to bf16, transpose
        for r in range(2):  # wq rows 256 -> 2 tiles
            wn = stage.tile([128, 1024], F32, tag="wstage")
            nc.sync.dma_start(wn[:], wq_ap[128 * r:128 * (r + 1), :])
            wb = bstage.tile([128, 1024], BF16, tag="wbst")
            nc.vector.tensor_copy(wb[:], wn[:])
            cast_transpose(wb[:], 8, lambda d, r=r: (wT[:, d, 128 * r:128 * (r + 1)], 128))
        for w_ap, col0 in ((wk_ap, 256), (wv_ap, 320)):
            wn = stage.tile([128, 1024], F32, tag="wstage")
            nc.sync.dma_start(wn[:64, :], w_ap[:, :])
            wb = bstage.tile([128, 1024], BF16, tag="wbst")
            nc.vector.tensor_copy(wb[:64, :], wn[:64, :])
            for dch in range(8):
                tr = trstp.tile([128, 512], BF16, tag="trst")
                nc.tensor.transpose(tr[:, :64], wb[:64, 128 * dch:128 * dch + 128],
                                    identb[:64, :64])
                nc.any.tensor_copy(wT[:, dch, col0:col0 + 64], tr[:, :64])
        for r in range(8):  # wo (1024, 256) -> 8 row tiles
            wn = stage.tile([128, 256], F32, tag="wostage")
            nc.sync.dma_start(wn[:], wo_ap[128 * r:128 * (r + 1), :])
            wb = bstage.tile([128, 256], BF16, tag="wobst")
            nc.vector.tensor_copy(wb[:], wn[:])
            cast_transpose(wb[:], 2, lambda oc, r=r: (woT[:, oc, 128 * r:128 * (r + 1)], 128))

        # ---- per t-block: load x, transpose, QKV proj, rope, Q/K transpose
        def phase12(tb):
            xs = stage.tile([128, 1024], F32, tag="xstage")
            nc.sync.dma_start(xs[:], x_ap[128 * tb:128 * (tb + 1), :])
            xb = bstage.tile([128, 1024], BF16, tag="xbst")
            nc.vector.tensor_copy(xb[:], xs[:])
            cast_transpose(xb[:], 8, lambda d, tb=tb: (xT[:, d, 128 * tb:128 * (tb + 1)], 128))
            qkv = qkvp.tile([128, 384], F32, tag="qkv")
            for dch in range(8):
                nc.tensor.matmul(qkv[:], xT[:, dch, 128 * tb:128 * (tb + 1)],
                                 wT[:, dch, :], start=(dch == 0), stop=(dch == 7))
            # V -> vaug (bf16)
            nc.any.tensor_copy(vaug[:, tb, 0:64], qkv[:, 320:384])
            # rope on Q(4 heads)+K(1 head) = 5 groups of 64
            qk = ropet.tile([128, 320], F32, tag="qknat")
            nc.scalar.copy(qk[:], qkv[:, 0:320])
            v4 = qk[:].rearrange("p (g i c) -> p g i c", g=5, c=2)
            re, im = v4[:, :, :, 0], v4[:, :, :, 1]
            rview = rope_sb[:][:, tb, :, :].rearrange("p g (i c) -> p g i c", c=2)
            cos_b, sin_b = rview[:, :, :, 0], rview[:, :, :, 1]
            t1 = ropet.tile([128, 5, 32], F32, tag="t1")
            t2 = ropet.tile([128, 5, 32], F32, tag="t2")
            t3 = ropet.tile([128, 5, 32], F32, tag="t3")
            t4 = ropet.tile([128, 5, 32], F32, tag="t4")
            nc.vector.tensor_mul(t1[:], re, cos_b)
            nc.vector.tensor_mul(t2[:], im, sin_b)
            nc.vector.tensor_mul(t3[:], re, sin_b)
            nc.vector.tensor_mul(t4[:], im, cos_b)
            qr = qrp.tile([128, 320], BF16, tag="qr")
            q4 = qr[:].rearrange("p (g i c) -> p g i c", g=5, c=2)
            nc.vector.tensor_sub(q4[:, :, :, 0], t1[:], t2[:])
            nc.vector.tensor_add(q4[:, :, :, 1], t3[:], t4[:])
            for h in range(4):
                tr = trstp.tile([128, 512], BF16, tag="trst")
                nc.tensor.transpose(tr[:64, :128], qr[:, 64 * h:64 * (h + 1)], identb[:])
                if h % 2 == 0:
                    nc.vector.tensor_copy(qT[:, h, 128 * tb:128 * (tb + 1)], tr[:64, :128])
                else:
                    nc.scalar.copy(qT[:, h, 128 * tb:128 * (tb + 1)], tr[:64, :128])
            tr = trstp.tile([128, 512], BF16, tag="trst")
            nc.tensor.transpose(tr[:64, :128], qr[:, 256:320], identb[:])
            nc.any.tensor_copy(kT[:, 128 * tb:128 * (tb + 1)], tr[:64, :128])

        partial = dram.tile([T, D], F32)

        # ---- attention for tq-slice j (tq 512j..512j+511), all 4 heads
        def phase3(j):
            onats = []
            for c in range(4):
                on = onatp.tile([128, OC], BF16, tag="onat")
                onats.append(on)
            for h in range(4):
                oas = [oap.tile([128, 65], F32, tag="oa", name=f"oa{_c}") for _c in range(4)]
                for i in range(4 * j + 4):
                    o0 = max(0, 128 * i - 512 * j)
                    st = trstp.tile([128, 512], F32, tag="trst")
                    nc.tensor.matmul(
                        st[:, o0:512],
                        kT[:, 128 * i:128 * (i + 1)],
                        qT[:, h, 512 * j + o0:512 * (j + 1)],
                        start=True, stop=True)
                    pt = ptp.tile([128, 512], BF16, tag="pt")
                    nc.scalar.activation(pt[:, o0:512], st[:, o0:512],
                                         mybir.ActivationFunctionType.Exp, scale=SCALE)
                    if i >= 4 * j:  # diagonal block: zero tq < tk after exp
                        c = i - 4 * j
                        nc.gpsimd.affine_select(
                            out=pt[:, 128 * c:128 * (c + 1)],
                            in_=pt[:, 128 * c:128 * (c + 1)],
                            compare_op=mybir.AluOpType.is_ge,
                            fill=0.0, base=0,
                            pattern=[[1, 128]], channel_multiplier=-1)
                    for c in range(4):
                        if i <= 4 * j + c:
                            nc.tensor.matmul(
                                oas[c][:],
                                pt[:, 128 * c:128 * (c + 1)],
                                vaug[:, i, :],
                                start=(i == 0), stop=(i == 4 * j + c))
                for c in range(4):
                    rc = rcp.tile([128, 1], F32, tag="rc")
                    nc.vector.reciprocal(rc[:], oas[c][:, 64:65])
                    nc.vector.tensor_scalar_mul(onats[c][:, DH * h:DH * (h + 1)],
                                                oas[c][:, 0:64], rc[:])
            # O transpose + output projection + partial store, per tq block
            for c in range(4):
                tb = 4 * j + c
                for oc in range(2):
                    tr = trstp.tile([128, 512], BF16, tag="trst")
                    nc.tensor.transpose(tr[:, :128],
                                        onats[c][:, 128 * oc:128 * (oc + 1)], identb[:])
                    if oc == 0:
                        nc.vector.tensor_copy(oT[:, oc, 128 * tb:128 * (tb + 1)], tr[:, :128])
                    else:
                        nc.scalar.copy(oT[:, oc, 128 * tb:128 * (tb + 1)], tr[:, :128])
                for ns in range(2):
                    op = trstp.tile([128, 512], F32, tag="trst", name=f"op{ns}")
                    for oc in range(2):
                        nc.tensor.matmul(op[:], oT[:, oc, 128 * tb:128 * (tb + 1)],
                                         woT[:, oc, 512 * ns:512 * (ns + 1)],
                                         start=(oc == 0), stop=(oc == 1))
                    ob = outsbp.tile([128, 512], F32, tag="outsb")
                    nc.vector.tensor_copy(ob[:], op[:])
                    nc.sync.dma_start(
                        partial[128 * tb:128 * (tb + 1), 512 * ns:512 * (ns + 1)], ob[:])

        for j in range(NJ):
            for tb in range(4 * j, 4 * j + 4):
                phase12(tb)
            phase3(j)
            # rows 512j..512j+512 complete -> ReduceScatter this quarter now
            rsout = dram.tile([128, D], F32, name=f"rsout{j}")
            nc.gpsimd.collective_compute(
                "ReduceScatter", mybir.AluOpType.add, replica_groups=GROUPS,
                ins=[partial[512 * j:512 * (j + 1), :].opt()],
                outs=[rsout.opt()])
            nc.sync.dma_start(out_ap[128 * j:128 * (j + 1), :], rsout[:])


def _build():
    if "nc" in _CACHE:
        return _CACHE["nc"]
    nc = bacc.Bacc("TRN2", target_bir_lowering=False, debug=False, num_devices=8)
    x_ap = nc.dram_tensor("x", [T, D], F32, kind="ExternalInput").ap()
    wq_ap = nc.dram_tensor("wq", [OC, D], F32, kind="ExternalInput").ap()
    wk_ap = nc.dram_tensor("wk", [DH, D], F32, kind="ExternalInput").ap()
    wv_ap = nc.dram_tensor("wv", [DH, D], F32, kind="ExternalInput").ap()
    wo_ap = nc.dram_tensor("wo", [D, OC], F32, kind="ExternalInput").ap()
    rope_ap = nc.dram_tensor("rope", [T, DH], F32, kind="ExternalInput").ap()
    out_ap = nc.dram_tensor("out", [T // 4, D], F32, kind="ExternalOutput").ap()
    with tile.TileContext(nc) as tc:
        _emit(nc, tc, (x_ap, wq_ap, wk_ap, wv_ap, wo_ap, rope_ap, out_ap))
    nc.compile()
    _CACHE["nc"] = nc
    return nc


def run(trace=False, tmpdir=None, **inputs):
    x = inputs["x"]
    rope2 = np.ascontiguousarray(
        inputs["rope_freqs"].astype(np.float32).reshape(T, DH))
    w_q, w_k, w_v, w_o = (np.asarray(inputs[k], np.float32)
                          for k in ("w_q", "w_k", "w_v", "w_o"))
    nc = _build()
    in_maps = []
    for c in range(8):
        b, g = divmod(c, 4)
        in_maps.append({
            "x": np.ascontiguousarray(x[b], dtype=np.float32),
            "wq": np.ascontiguousarray(w_q[OC * g:OC * (g + 1)]),
            "wk": np.ascontiguousarray(w_k[DH * g:DH * (g + 1)]),
            "wv": np.ascontiguousarray(w_v[DH * g:DH * (g + 1)]),
            "wo": np.ascontiguousarray(w_o[:, OC * g:OC * (g + 1)]),
            "rope": rope2,
        })
    res = run_bass_kernel_spmd(nc, in_maps, core_ids=list(range(8)), trace=trace,
                               tmpdir=tmpdir)
    out = np.empty((B, T, D), np.float32)
    for core in range(8):
        b, r = divmod(core, 4)
        for c in range(4):
            out[b, 512 * c + 128 * r:512 * c + 128 * (r + 1)] = \
                res.results[core]["out"][128 * c:128 * (c + 1)]
    return out, res


def kernel(**inputs):
    out, _ = run(trace=False, **inputs)
    return out

